# revision 8
# baseline (speedup 1.0000x reference)
"""AdditiveAttention (d2l-style) Trainium2 kernel, SPMD over 8 NeuronCores.

Problem shapes: B=16, Q=64, K=1024, DQ=DK=256, H=128, DV=256.

Sharding: data-parallel over the QUERY axis (8 queries per core), with every
core processing all 16 batches. This keeps the single SPMD instruction
stream identical across cores while allowing the graph (built at runtime
from the actual valid_lens values) to skip fully-invalid key tiles — a
large win since valid_lens average ~K/2.

Per-core pipeline:
  - host packs keys^T (valid 128-tiles only) and [values | ones] as fp16
  - PE: kproj^T = Wk^T @ keys^T per tile; qproj^T likewise
  - ACT: feat = tanh(kproj^T + qproj_col) with qproj as per-partition bias
  - PE: scores col = feat_tile^T @ wv  (into one [128k, T*8] PSUM tile,
        already transposed for the attention@V matmul; PE outputs must
        start at partition 0/32/64, so row-major scores are not an option)
  - ACT: e = exp(scores)  (no max-subtract needed; |scores| <~ sum|wv|)
  - PE: [out | denom] = e_task^T @ [V | 1] accumulated over valid tiles,
        partial-tile rows zeroed in e first
  - DVE: out = out * (1/denom) ; DMA to DRAM
"""

import sys

sys.path.insert(0, "/opt/trn_rl_repo")

from contextlib import ExitStack

import numpy as np

import concourse.bass as bass  # noqa: F401  (bass must import before tile)
import concourse.tile as tile
from concourse import bacc, masks, mybir
from concourse.bass_utils import run_bass_kernel_spmd

B, Q, KLEN, D, H, DV = 16, 64, 1024, 256, 128, 256
N_CORES = 8
QS = Q // N_CORES  # queries per core = 8
KT = 128  # key tile size

F16 = mybir.dt.float16
F32 = mybir.dt.float32


def _build_graph(tiles, vls):
    """tiles[b] = number of valid 128-key tiles for batch b; vls[b] = valid_lens[b]."""
    T = int(sum(tiles))
    offs = np.concatenate([[0], np.cumsum(tiles)]).astype(int)

    nc = bacc.Bacc("TRN2", target_bir_lowering=False, debug=False, num_devices=N_CORES)
    kT_d = nc.dram_tensor("kT", [T, 2, KT, KT], F16, kind="ExternalInput").ap()
    v1_d = nc.dram_tensor("v1", [T, KT, DV + 1], F16, kind="ExternalInput").ap()
    qT_d = nc.dram_tensor("qT", [2, KT, B * QS], F16, kind="ExternalInput").ap()
    wq_d = nc.dram_tensor("wqc", [2, KT, H], F16, kind="ExternalInput").ap()
    wk_d = nc.dram_tensor("wkc", [2, KT, H], F16, kind="ExternalInput").ap()
    wv_d = nc.dram_tensor("wv", [H, 1], F16, kind="ExternalInput").ap()
    out_d = nc.dram_tensor("out", [B * QS, DV], F32, kind="ExternalOutput").ap()

    with tile.TileContext(nc) as tc, ExitStack() as ctx:
        const = ctx.enter_context(tc.tile_pool(name="const", bufs=1))
        spool = ctx.enter_context(tc.tile_pool(name="s_psum", bufs=1, space="PSUM"))
        epool = ctx.enter_context(tc.tile_pool(name="e_sb", bufs=1))
        vpool = ctx.enter_context(tc.tile_pool(name="vals", bufs=T))
        fin = ctx.enter_context(tc.tile_pool(name="fin", bufs=1))

        # constants
        wq_sb = [const.tile([KT, H], F16, tag=f"wq{c}", name=f"wq{c}") for c in range(2)]
        wk_sb = [const.tile([KT, H], F16, tag=f"wk{c}", name=f"wk{c}") for c in range(2)]
        qT_sb = [const.tile([KT, B * QS], F16, tag=f"qt{c}", name=f"qt{c}") for c in range(2)]
        for c in range(2):
            nc.sync.dma_start(wq_sb[c][:], wq_d[c])
            nc.sync.dma_start(wk_sb[c][:], wk_d[c])
            nc.sync.dma_start(qT_sb[c][:], qT_d[c])
        wv_sb = const.tile([H, 1], F16, tag="wv", name="wv_sb")
        nc.sync.dma_start(wv_sb[:], wv_d[:])
        qproj_sb = const.tile([H, B * QS], F32, tag="qproj", name="qproj_sb")

        # transposed scores: partition = key-in-tile, free = task*QS + j
        S = spool.tile([KT, T * QS], F32)

        # ---- phase 0: query projection ----
        with tc.tile_pool(name="qp_psum", bufs=1, space="PSUM") as qpp:
            qp = qpp.tile([H, B * QS], F32)
            for c in range(2):
                nc.tensor.matmul(
                    qp[:], wq_sb[c][:], qT_sb[c][:], start=(c == 0), stop=(c == 1)
                )
            nc.vector.tensor_copy(qproj_sb[:], qp[:])

        vals_sb = []

        # ---- phase 1: kproj, tanh features, score matvecs ----
        with (
            tc.tile_pool(name="kt_sb", bufs=6) as ktp,
            tc.tile_pool(name="kp_psum", bufs=2, space="PSUM") as kpp,
            tc.tile_pool(name="kproj_sb", bufs=2) as kjp,
            tc.tile_pool(name="feat", bufs=3) as fp,
        ):
            for b in range(B):
                nt = int(tiles[b])
                vlpad = nt * KT
                kp = kpp.tile([H, KLEN], F32, tag="kp", name="kp")
                for t in range(nt):
                    task = int(offs[b]) + t
                    # prefetch the [V | 1] tile for phase 2
                    vt = vpool.tile([KT, DV + 1], F16, tag="v1", name="vt")
                    nc.sync.dma_start(vt[:], v1_d[task])
                    vals_sb.append(vt)
                    k0 = ktp.tile([KT, KT], F16, tag="k0", name="k0")
                    k1 = ktp.tile([KT, KT], F16, tag="k1", name="k1")
                    nc.sync.dma_start(k0[:], kT_d[task, 0])
                    nc.sync.dma_start(k1[:], kT_d[task, 1])
                    sl = kp[:, t * KT : (t + 1) * KT]
                    nc.tensor.matmul(sl, wk_sb[0][:], k0[:], start=True, stop=False)
                    nc.tensor.matmul(sl, wk_sb[1][:], k1[:], start=False, stop=True)
                kproj = kjp.tile([H, KLEN], F16, tag="kproj", name="kproj")
                nc.vector.tensor_copy(kproj[:, :vlpad], kp[:, :vlpad])
                for j in range(QS):
                    bq = b * QS + j
                    feat = fp.tile([H, KLEN], F16, tag="feat", name="feat")
                    nc.scalar.activation(
                        feat[:, :vlpad],
                        kproj[:, :vlpad],
                        mybir.ActivationFunctionType.Tanh,
                        bias=qproj_sb[:, bq : bq + 1],
                    )
                    for t in range(nt):
                        gcol = (int(offs[b]) + t) * QS + j
                        nc.tensor.matmul(
                            S[:, gcol : gcol + 1],
                            feat[:, t * KT : (t + 1) * KT],
                            wv_sb[:],
                            start=True,
                            stop=True,
                        )

        # ---- phase 2: exp, attention @ [V | 1] ----
        # invalid key rows of each batch's final partial tile are handled by
        # host-side zeroing of the corresponding [V | 1] rows, so e needs no
        # masking here.
        e = epool.tile([KT, T * QS], F16)
        nc.scalar.activation(e[:, :], S[:, :], mybir.ActivationFunctionType.Exp)

        with (
            tc.tile_pool(name="o_psum", bufs=3, space="PSUM") as op,
        ):
            for b in range(B):
                nt = int(tiles[b])
                Ob = op.tile([QS, DV + 1], F32, tag="ob", name="Ob")
                for t in range(nt):
                    task = int(offs[b]) + t
                    nc.tensor.matmul(
                        Ob[:],
                        e[:, task * QS : (task + 1) * QS],
                        vals_sb[task][:],
                        start=(t == 0),
                        stop=(t == nt - 1),
                    )
                recip = fin.tile([QS, 1], F32, tag="recip", name="recip", bufs=2)
                nc.vector.reciprocal(recip[:], Ob[:, DV : DV + 1])
                outf = fin.tile([QS, DV], F32, tag="outf", name="outf", bufs=3)
                nc.vector.tensor_scalar_mul(outf[:], Ob[:, :DV], recip[:])
                nc.sync.dma_start(out_d[b * QS : (b + 1) * QS, :], outf[:])

    nc.compile()
    return nc


def kernel(queries, keys, values, valid_lens, Wq, Wk, wv):
    queries = np.asarray(queries, dtype=np.float32)
    keys = np.asarray(keys, dtype=np.float32)
    values = np.asarray(values, dtype=np.float32)
    vl = np.asarray(valid_lens).astype(np.int64)
    Wq = np.asarray(Wq, dtype=np.float32)
    Wk = np.asarray(Wk, dtype=np.float32)
    wv = np.asarray(wv, dtype=np.float32)

    tiles = np.maximum(1, -(-vl // KT))  # ceil, >=1
    T = int(tiles.sum())

    # packed keys^T: [T, 2, 128, 128] fp16 (d-chunk, d, k)
    kT_pack = np.empty((T, 2, KT, KT), dtype=np.float16)
    v1_pack = np.empty((T, KT, DV + 1), dtype=np.float16)
    v1_pack[:, :, DV] = 1.0
    i = 0
    for b in range(B):
        nt = int(tiles[b])
        for t in range(nt):
            ksl = keys[b, t * KT : (t + 1) * KT, :]  # [128, 256]
            kT_pack[i] = ksl.T.reshape(2, KT, KT).astype(np.float16)
            v1_pack[i, :, :DV] = values[b, t * KT : (t + 1) * KT, :]
            if t == nt - 1:
                rows = int(vl[b]) - t * KT
                v1_pack[i, rows:, :] = 0.0  # mask invalid keys via V and ones col
            i += 1

    wqc = Wq.reshape(2, KT, H).astype(np.float16)
    wkc = Wk.reshape(2, KT, H).astype(np.float16)
    wv_c = wv.reshape(H, 1).astype(np.float16)

    nc = _build_graph(tiles, vl)

    in_maps = []
    for c in range(N_CORES):
        qc = queries[:, c * QS : (c + 1) * QS, :].reshape(B * QS, D)
        qT = np.ascontiguousarray(qc.T).reshape(2, KT, B * QS).astype(np.float16)
        in_maps.append(
            {
                "kT": kT_pack,
                "v1": v1_pack,
                "qT": qT,
                "wqc": wqc,
                "wkc": wkc,
                "wv": wv_c,
            }
        )

    res = run_bass_kernel_spmd(nc, in_maps, core_ids=list(range(N_CORES)))

    out = np.empty((B, Q, DV), dtype=np.float32)
    for c in range(N_CORES):
        out[:, c * QS : (c + 1) * QS, :] = res.results[c]["out"].reshape(B, QS, DV)
    return out
